# revision 3
# baseline (speedup 1.0000x reference)
"""LConv (7x7 position-linear conv) Trainium2 Bass kernel.

Full inputs in, full output out. Sharding: data-parallel over batch,
16 images -> 8 NeuronCores (2 images/core). abc/bias replicated.

Math: the 7x7 kernel weight is linear in position:
  w[u,v,c,o] = u*A[c,o] + v*B[c,o] + C[c,o]   (u,v in -3..3)
so with R = box7 along W of x, Q = box7 along H of x:
  out[o,i,j] = sum_u (u*A + C)[.,o] . R[., i+u, j]
             + sum_{v!=0} (v*B)[.,o] . Q[., i, j+v]  + bias[o]
i.e. 13 matmul taps per output tile over just TWO box-filtered maps.

R: one tensor_tensor_scan per row-chunk (sliding-box recurrence
   state += x[n+7] - x[n]; 7 leading zero cols per row make any
   row-aligned chunk self-contained).
Q: one custom-DVE pass (cumsum(in0)-cumsum(in1)) over a column-major
   view; 7 lead + 7 trail zero ROWS make every column page
   self-telescoping.

Layout: padded grid PH2 x PW2 = 126 x 122 per image;
  rows: 7 lead zeros + 112 data + 7 trail zeros
  cols: 7 lead zeros + 112 data + 3 trail zeros
R is grid-aligned (value for data(i,j) at grid (7+i, 3+j));
Q is column-major (page p = grid col 4+p, offset k = i+3).
"""

import numpy as np

import concourse.bacc as bacc
import concourse.mybir as mybir
from concourse import tile
from concourse.bass_utils import run_bass_kernel_spmd

F32 = mybir.dt.float32
BF16 = mybir.dt.bfloat16
AF = mybir.ActivationFunctionType
ALU = mybir.AluOpType

B_TOT = 16
N_CORES = 8
B_PER = B_TOT // N_CORES
CIN = 128
COUT = 128
H = W = 112
PW2 = 122                  # 7 lead + 112 + 3 trail cols
PH2 = 126                  # 7 lead + 112 + 7 trail rows
XBF = PH2 * PW2            # 15372
QK = 119                   # Q per-page length (box rows k+1..k+7)
QP = 118                   # Q pages = grid cols 4..121
QSCAN_P = 115              # scanned pages = grid cols 4..118
NCHUNK = 4
CROWS = H // NCHUNK        # 28 rows per chunk
CLEN = CROWS * PW2         # 3416
CSCAN = CLEN - 7           # 3409: keep reads inside the chunk
OUT_ROWS = 4
OTF = OUT_ROWS * W         # 448
TPG = 7                    # psum tiles per group
GROUPS = H // (OUT_ROWS * TPG)   # 4
QTAP_V = (-3, -2, -1, 1, 2, 3)
NTAPS = 7 + len(QTAP_V)    # 13

_CACHE = {}


def _register_opa():
    from concourse.dve_spec import Spec, Src0, Src1, scan, AluOp, lower
    import concourse.dve_ops as dve_ops
    from concourse.dve_uop import DveOpSpec

    if any(op.name == "BOXDIFF7" for op in dve_ops.OPS):
        return next(op for op in dve_ops.OPS if op.name == "BOXDIFF7")
    spec = Spec(
        body=scan(AluOp.ADD, Src0) - scan(AluOp.ADD, Src1),
        reference=lambda in0, in1: (
            np.cumsum(in0, axis=-1) - np.cumsum(in1, axis=-1)
        ),
    )
    row = dve_ops._CUSTOM_DVE_ROW_BASE + len(dve_ops.OPS)
    shas = {}
    for ver in ("v3", "v4"):
        s = DveOpSpec(
            name="BOXDIFF7", opcode=row, uops=lower(spec, ver=ver), rd1_en=True
        )
        shas[ver] = s.sha(ver)
    op = dve_ops.DveOp("BOXDIFF7", spec, subdim=False, uops_sha=shas)
    dve_ops.OPS.append(op)
    dve_ops._SUB_OPCODE_FOR_NAME[op.name] = row
    dve_ops.CUSTOM_DVE_SPECS[op.name] = op.spec
    return op


def _build():
    nc = bacc.Bacc("TRN2", target_bir_lowering=False, debug=False)
    opa = _register_opa()

    t_x = nc.dram_tensor("xs", [B_PER, CIN, H, W], F32, kind="ExternalInput")
    t_w = nc.dram_tensor("wts", [NTAPS, CIN, COUT], F32, kind="ExternalInput")
    t_bias = nc.dram_tensor("bias", [COUT, 1], F32, kind="ExternalInput")
    t_out = nc.dram_tensor("out", [B_PER, COUT, H, W], BF16, kind="ExternalOutput")

    with tile.TileContext(nc) as tc:
        with (
            tc.tile_pool(name="const", bufs=1) as cpool,
            tc.tile_pool(name="xb", bufs=1) as xpool,
            tc.tile_pool(name="maps", bufs=1) as mpool,
            tc.tile_pool(name="outs", bufs=4) as opool,
            tc.tile_pool(name="ps", bufs=1, space="PSUM") as ppool,
        ):
            # ---- constants ----
            wf = cpool.tile([CIN, NTAPS * COUT], F32, tag="wf")
            nc.sync.dma_start(
                wf[:].rearrange("c (t o) -> c t o", t=NTAPS),
                t_w[:].transpose([1, 0, 2]),
            )
            wt = cpool.tile([CIN, NTAPS * COUT], BF16, tag="wt")
            nc.vector.tensor_copy(wt[:], wf[:])
            bias_sb = cpool.tile([COUT, 1], F32, tag="bias")
            nc.sync.dma_start(bias_sb[:], t_bias[:])

            # ---- buffers ----
            xb = xpool.tile([CIN, XBF], F32, tag="xb")
            xbv = xb[:].rearrange("c (r q) -> c r q", q=PW2)
            xbt = xb[:].rearrange("c (r q) -> c q r", q=PW2)  # [c,122,126]
            # zero the pad regions (data region is DMA-overwritten per image)
            nc.vector.memset(xb[:, : 7 * PW2], 0.0)
            nc.vector.memset(xb[:, (7 + H) * PW2 :], 0.0)
            nc.vector.memset(xbv[:, 7 : 7 + H, 0:7], 0.0)
            nc.vector.memset(xbv[:, 7 : 7 + H, 7 + W :], 0.0)

            rbufs, qbufs = [], []
            for i in range(2):
                r = mpool.tile([CIN, XBF], BF16, tag=f"R{i}")
                nc.vector.memset(r[:, : 7 * PW2], 0.0)
                nc.vector.memset(r[:, (7 + H) * PW2 :], 0.0)
                rbufs.append(r)
                q = mpool.tile([CIN, QP * QK], BF16, tag=f"Q{i}")
                nc.vector.memset(q[:, QSCAN_P * QK :], 0.0)
                qbufs.append(q)

            for b in range(B_PER):
                R, Q = rbufs[b % 2], qbufs[b % 2]
                rv = R[:].rearrange("c (r q) -> c r q", q=PW2)
                qt = Q[:].rearrange("c (p k) -> c k p", k=QK)  # [c,119,118]

                # ---- load image (4 row-chunks) ----
                for ch in range(NCHUNK):
                    r0 = ch * CROWS
                    nc.sync.dma_start(
                        xbv[:, 7 + r0 : 7 + r0 + CROWS, 7 : 7 + W],
                        t_x[b, :, r0 : r0 + CROWS, :],
                    )

                # ---- R = box7 along W (chunk 0 first so PE starts early) --
                def r_chunk(ch):
                    base = (7 + ch * CROWS) * PW2
                    nc.vector.tensor_tensor_scan(
                        R[:, base : base + CSCAN],
                        xb[:, base + 7 : base + 7 + CSCAN],
                        xb[:, base : base + CSCAN],
                        0.0,
                        op0=ALU.add,
                        op1=ALU.subtract,
                    )

                r_chunk(0)
                # ---- Q = box7 along H (one column-major custom scan) ----
                nc.vector._custom_dve(
                    opa,
                    out=Q[:, : QSCAN_P * QK].rearrange("c (p k) -> c p k", k=QK),
                    in0=xbt[:, 4 : 4 + QSCAN_P, 7:126],
                    in1=xbt[:, 4 : 4 + QSCAN_P, 0:119],
                )
                for ch in range(1, NCHUNK):
                    r_chunk(ch)

                # ---- 13-tap matmuls, weight-major per 7-tile group ----
                for g in range(GROUPS):
                    accs = [
                        ppool.tile([COUT, OTF], F32, tag=f"acc{t}", name=f"acc{t}")
                        for t in range(TPG)
                    ]
                    for tap in range(NTAPS):
                        wslice = wt[:, tap * COUT : (tap + 1) * COUT]
                        for t in range(TPG):
                            i0 = (g * TPG + t) * OUT_ROWS
                            if tap < 7:
                                u = tap - 3
                                rhs = rv[:, 7 + i0 + u : 7 + i0 + u + 4, 3 : 3 + W]
                            else:
                                v = QTAP_V[tap - 7]
                                rhs = qt[:, i0 + 3 : i0 + 7, 3 + v : 3 + v + W]
                            nc.tensor.matmul(
                                accs[t][:],
                                wslice,
                                rhs,
                                start=(tap == 0),
                                stop=(tap == NTAPS - 1),
                            )
                    for t in range(TPG):
                        i0 = (g * TPG + t) * OUT_ROWS
                        ot = opool.tile([COUT, OTF], BF16, tag="ot")
                        nc.scalar.activation(
                            ot[:], accs[t][:], AF.Identity, bias=bias_sb[:], scale=1.0
                        )
                        nc.sync.dma_start(
                            t_out[b, :, i0 : i0 + OUT_ROWS, :].rearrange(
                                "o r j -> o (r j)"
                            ),
                            ot[:],
                        )

    nc.compile()
    return nc


def _make_in_maps(x, abc, bias):
    A, Bm, Cc = abc[0:128], abc[128:256], abc[256:384]
    taps = [u * A + Cc for u in range(-3, 4)] + [v * Bm for v in QTAP_V]
    wts = np.ascontiguousarray(np.stack(taps), dtype=np.float32)
    bias2 = np.ascontiguousarray(bias.reshape(COUT, 1), dtype=np.float32)
    return [
        {
            "xs": np.ascontiguousarray(x[c * B_PER : (c + 1) * B_PER]),
            "wts": wts,
            "bias": bias2,
        }
        for c in range(N_CORES)
    ]


def kernel(x: np.ndarray, abc: np.ndarray, bias: np.ndarray) -> np.ndarray:
    x = np.ascontiguousarray(x, dtype=np.float32)
    abc = np.asarray(abc, dtype=np.float32)
    bias = np.asarray(bias, dtype=np.float32)

    if "nc" not in _CACHE:
        _CACHE["nc"] = _build()
    nc = _CACHE["nc"]

    in_maps = _make_in_maps(x, abc, bias)
    res = run_bass_kernel_spmd(nc, in_maps, list(range(N_CORES)))
    out = np.concatenate(
        [np.asarray(res.results[c]["out"]) for c in range(N_CORES)], axis=0
    )
    return out.astype(np.float32)


if __name__ == "__main__":
    rng = np.random.default_rng(0)
    x = rng.standard_normal((16, 128, 112, 112), dtype=np.float32)
    abc = (rng.standard_normal((384, 128)) * 0.05).astype(np.float32)
    bias = (rng.standard_normal((128,)) * 0.05).astype(np.float32)
    out = kernel(x=x, abc=abc, bias=bias)
    print(out.shape, out.dtype)


# revision 5
# speedup vs baseline: 1.8431x; 1.8431x over previous
"""LConv (7x7 position-linear conv) Trainium2 Bass kernel.

Full inputs in, full output out. Sharding: data-parallel over batch,
16 images -> 8 NeuronCores (2 images/core). abc/bias replicated.

Math: the 7x7 kernel weight is linear in position:
  w[u,v,c,o] = u*A[c,o] + v*B[c,o] + C[c,o]   (u,v in -3..3)
so with R = box7 along W of x and Q = box7 along H of x:
  out[o,i,j] = sum_u (u*A + C)[.,o] . R[., i+u, j]
             + sum_{v!=0} (v*B)[.,o] . Q[., i, j+v]  + bias[o]
13 matmul taps per 4-row output tile over two box-filtered maps; all
rhs views are row-contiguous (PE streams at ~N cycles per matmul).

R: sliding-box via the BOXDIFF custom-DVE op (cumsum(in0)-cumsum(in1))
   on the row-major stream; 7 lead zero cols per row make any
   row-aligned chunk self-contained.
Q: same op on a column-major (transposed) view, writing into a
   row-major Q buffer; 7 lead + 7 trail zero rows make every column
   page self-telescoping.

The image pair is processed as 4 half-image units (56 out rows each,
+/-3-row halo) so DMA, scans, and matmuls pipeline across units;
unit slot parity == top/bottom parity, so the static zero borders per
slot are set up once.
"""

import numpy as np

import concourse.bacc as bacc
import concourse.mybir as mybir
from concourse import tile
from concourse.bass_utils import run_bass_kernel_spmd

F32 = mybir.dt.float32
BF16 = mybir.dt.bfloat16
AF = mybir.ActivationFunctionType
ALU = mybir.AluOpType

B_TOT = 16
N_CORES = 8
B_PER = B_TOT // N_CORES
CIN = 128
COUT = 128
H = W = 112
PW2 = 122                  # 7 lead + 112 + 3 trail cols
UROWS = 56                 # output rows per unit (half image)
XROWS = 76                 # 7 lead + 62 (56+halo) + 7 trail rows
XBF = XROWS * PW2          # 9272
DROWS = 59                 # valid x rows DMA'd per unit
QK = XROWS - 7             # 69 scanned values per column page
OUT_ROWS = 4
OTF = OUT_ROWS * W         # 448
TPG = 7                    # psum tiles per group
GPU = UROWS // (OUT_ROWS * TPG)  # 2 groups per unit
QTAP_V = (-3, -2, -1, 1, 2, 3)
NTAPS = 7 + len(QTAP_V)    # 13
# R-scan chunks (row-aligned; chunk 0 covers all rows group 1 needs)
RCHUNKS = ((7, 34), (41, 28))

_CACHE = {}


def _register_opa():
    from concourse.dve_spec import Spec, Src0, Src1, scan, AluOp, lower
    import concourse.dve_ops as dve_ops
    from concourse.dve_uop import DveOpSpec

    if any(op.name == "BOXDIFF7" for op in dve_ops.OPS):
        return next(op for op in dve_ops.OPS if op.name == "BOXDIFF7")
    spec = Spec(
        body=scan(AluOp.ADD, Src0) - scan(AluOp.ADD, Src1),
        reference=lambda in0, in1: (
            np.cumsum(in0, axis=-1) - np.cumsum(in1, axis=-1)
        ),
    )
    row = dve_ops._CUSTOM_DVE_ROW_BASE + len(dve_ops.OPS)
    shas = {}
    for ver in ("v3", "v4"):
        s = DveOpSpec(
            name="BOXDIFF7", opcode=row, uops=lower(spec, ver=ver), rd1_en=True
        )
        shas[ver] = s.sha(ver)
    op = dve_ops.DveOp("BOXDIFF7", spec, subdim=False, uops_sha=shas)
    dve_ops.OPS.append(op)
    dve_ops._SUB_OPCODE_FOR_NAME[op.name] = row
    dve_ops.CUSTOM_DVE_SPECS[op.name] = op.spec
    return op


def _build():
    nc = bacc.Bacc("TRN2", target_bir_lowering=False, debug=False)
    opa = _register_opa()

    t_x = nc.dram_tensor("xs", [B_PER, CIN, H, W], F32, kind="ExternalInput")
    t_w = nc.dram_tensor("wts", [NTAPS, CIN, COUT], F32, kind="ExternalInput")
    t_bias = nc.dram_tensor("bias", [COUT, 1], F32, kind="ExternalInput")
    t_out = nc.dram_tensor("out", [B_PER, COUT, H, W], BF16, kind="ExternalOutput")

    with tile.TileContext(nc) as tc:
        with (
            tc.tile_pool(name="const", bufs=1) as cpool,
            tc.tile_pool(name="bufs", bufs=1) as bpool,
            tc.tile_pool(name="outs", bufs=4) as opool,
            tc.tile_pool(name="ps", bufs=1, space="PSUM") as ppool,
        ):
            # ---- constants ----
            wf = cpool.tile([CIN, NTAPS * COUT], F32, tag="wf")
            nc.sync.dma_start(
                wf[:].rearrange("c (t o) -> c t o", t=NTAPS),
                t_w[:].transpose([1, 0, 2]),
            )
            wt = cpool.tile([CIN, NTAPS * COUT], BF16, tag="wt")
            nc.vector.tensor_copy(wt[:], wf[:])
            bias_sb = cpool.tile([COUT, 1], F32, tag="bias")
            nc.sync.dma_start(bias_sb[:], t_bias[:])

            # ---- per-slot buffers (slot = unit parity = top/bottom) ----
            xbufs, rbufs, qbufs = [], [], []
            for s in range(2):
                xb = bpool.tile([CIN, XBF], F32, tag=f"xb{s}", name=f"xb{s}")
                xv = xb[:].rearrange("c (r q) -> c r q", q=PW2)
                nc.vector.memset(xb[:, : 7 * PW2], 0.0)          # lead rows
                nc.vector.memset(xb[:, (XROWS - 7) * PW2 :], 0.0)  # trail rows
                nc.vector.memset(xv[:, 7 : XROWS - 7, 0:7], 0.0)   # lead cols
                nc.vector.memset(xv[:, 7 : XROWS - 7, 7 + W :], 0.0)  # trail cols
                if s == 0:
                    nc.vector.memset(xv[:, 7:10, :], 0.0)   # above-image pad
                else:
                    nc.vector.memset(xv[:, 66:69, :], 0.0)  # below-image pad
                xbufs.append(xb)
                r = bpool.tile([CIN, XBF], BF16, tag=f"R{s}", name=f"R{s}")
                rbufs.append(r)
                q = bpool.tile([CIN, XBF], BF16, tag=f"Q{s}", name=f"Q{s}")
                qv = q[:].rearrange("c (k g) -> c k g", g=PW2)
                nc.vector.memset(qv[:, :, 7 + W :], 0.0)  # right zero cols
                qbufs.append(q)

            for unit in range(B_PER * 2):
                b, hh = unit // 2, unit % 2
                xb, R, Q = xbufs[hh], rbufs[hh], qbufs[hh]
                xv = xb[:].rearrange("c (r q) -> c r q", q=PW2)
                xt = xb[:].rearrange("c (r q) -> c q r", q=PW2)  # [c,122,76]
                rv = R[:].rearrange("c (r q) -> c r q", q=PW2)
                qv = Q[:].rearrange("c (k g) -> c k g", g=PW2)
                qt = Q[:].rearrange("c (k g) -> c g k", g=PW2)   # [c,122,76]

                # ---- load unit rows (2 chunks aligned with R chunks) ----
                # x rows d0-3..d0+58 clipped to the image; slot parity fixes
                # the destination row offset.
                xr0 = max(0, 56 * hh - 3)            # first valid x row
                dst0 = 10 if hh == 0 else 7          # its XB row
                # chunk split at XB row 41 (exclusive)
                n0 = 41 - dst0
                nc.sync.dma_start(
                    xv[:, dst0 : dst0 + n0, 7 : 7 + W],
                    t_x[b, :, xr0 : xr0 + n0, :],
                )
                n1 = DROWS - n0
                nc.sync.dma_start(
                    xv[:, 41 : 41 + n1, 7 : 7 + W],
                    t_x[b, :, xr0 + n0 : xr0 + DROWS, :],
                )

                # ---- R chunk 0, Q scan, R chunk 1 ----
                def r_chunk(r0, nrows):
                    base = r0 * PW2
                    ln = nrows * PW2 - 7
                    nc.vector._custom_dve(
                        opa,
                        out=R[:, base : base + ln],
                        in0=xb[:, base + 7 : base + 7 + ln],
                        in1=xb[:, base : base + ln],
                    )

                r_chunk(*RCHUNKS[0])
                nc.vector._custom_dve(
                    opa,
                    out=qt[:, 4 : 4 + 115, 0:QK],
                    in0=xt[:, 4 : 4 + 115, 7:XROWS],
                    in1=xt[:, 4 : 4 + 115, 0:QK],
                )
                r_chunk(*RCHUNKS[1])

                # ---- 13-tap matmuls, weight-major per 7-tile group ----
                for g in range(GPU):
                    accs = [
                        ppool.tile([COUT, OTF], F32, tag=f"acc{t}", name=f"acc{t}")
                        for t in range(TPG)
                    ]
                    for tap in range(NTAPS):
                        wslice = wt[:, tap * COUT : (tap + 1) * COUT]
                        for t in range(TPG):
                            i0 = (g * TPG + t) * OUT_ROWS  # unit-local out row
                            if tap < 7:
                                u = tap - 3
                                rhs = rv[:, 10 + i0 + u : 14 + i0 + u, 3 : 3 + W]
                            else:
                                v = QTAP_V[tap - 7]
                                rhs = qv[:, i0 + 6 : i0 + 10, 7 + v : 7 + v + W]
                            nc.tensor.matmul(
                                accs[t][:],
                                wslice,
                                rhs,
                                start=(tap == 0),
                                stop=(tap == NTAPS - 1),
                            )
                    for t in range(TPG):
                        i0 = (g * TPG + t) * OUT_ROWS
                        ot = opool.tile([COUT, OTF], BF16, tag="ot", name="ot")
                        nc.scalar.activation(
                            ot[:], accs[t][:], AF.Identity, bias=bias_sb[:], scale=1.0
                        )
                        nc.sync.dma_start(
                            t_out[
                                b, :, 56 * hh + i0 : 56 * hh + i0 + OUT_ROWS, :
                            ].rearrange("o r j -> o (r j)"),
                            ot[:],
                        )

    nc.compile()
    return nc


def _make_in_maps(x, abc, bias):
    A, Bm, Cc = abc[0:128], abc[128:256], abc[256:384]
    taps = [u * A + Cc for u in range(-3, 4)] + [v * Bm for v in QTAP_V]
    wts = np.ascontiguousarray(np.stack(taps), dtype=np.float32)
    bias2 = np.ascontiguousarray(bias.reshape(COUT, 1), dtype=np.float32)
    return [
        {
            "xs": np.ascontiguousarray(x[c * B_PER : (c + 1) * B_PER]),
            "wts": wts,
            "bias": bias2,
        }
        for c in range(N_CORES)
    ]


def kernel(x: np.ndarray, abc: np.ndarray, bias: np.ndarray) -> np.ndarray:
    x = np.ascontiguousarray(x, dtype=np.float32)
    abc = np.asarray(abc, dtype=np.float32)
    bias = np.asarray(bias, dtype=np.float32)

    if "nc" not in _CACHE:
        _CACHE["nc"] = _build()
    nc = _CACHE["nc"]

    in_maps = _make_in_maps(x, abc, bias)
    res = run_bass_kernel_spmd(nc, in_maps, list(range(N_CORES)))
    out = np.concatenate(
        [np.asarray(res.results[c]["out"]) for c in range(N_CORES)], axis=0
    )
    return out.astype(np.float32)


if __name__ == "__main__":
    rng = np.random.default_rng(0)
    x = rng.standard_normal((16, 128, 112, 112), dtype=np.float32)
    abc = (rng.standard_normal((384, 128)) * 0.05).astype(np.float32)
    bias = (rng.standard_normal((128,)) * 0.05).astype(np.float32)
    out = kernel(x=x, abc=abc, bias=bias)
    print(out.shape, out.dtype)


# revision 9
# speedup vs baseline: 1.9546x; 1.0605x over previous
"""LConv (7x7 position-linear conv) Trainium2 Bass kernel.

Full inputs in, full output out. Sharding: data-parallel over batch,
16 images -> 8 NeuronCores (2 images/core). abc/bias replicated.

Math: the 7x7 kernel weight is linear in position:
  w[u,v,c,o] = u*A[c,o] + v*B[c,o] + C[c,o]   (u,v in -3..3)
so with R = box7 along W of x and Q = box7 along H of x:
  out[o,i,j] = sum_u (u*A + C)[.,o] . R[., i+u, j]
             + sum_{v!=0} (v*B)[.,o] . Q[., i, j+v]  + bias[o]
13 matmul taps per 4-row output tile over two box-filtered maps; all
rhs views are row-contiguous (PE streams at ~N cycles per matmul).

R: sliding-box via the BOXDIFF custom-DVE op (cumsum(in0)-cumsum(in1))
   on the row-major stream; 7 lead zero cols per row make any
   row-aligned chunk self-contained.
Q: same op on a column-major (transposed) view, writing into a
   row-major Q buffer; 7 lead + 7 trail zero rows make every column
   page self-telescoping.

The image pair is processed as 4 half-image units (56 out rows each,
+/-3-row halo) so DMA, scans, and matmuls pipeline across units;
unit slot parity == top/bottom parity, so the static zero borders per
slot are set up once.
"""

import numpy as np

import concourse.bacc as bacc
import concourse.mybir as mybir
from concourse import tile
from concourse.bass_utils import run_bass_kernel_spmd

F32 = mybir.dt.float32
BF16 = mybir.dt.bfloat16
AF = mybir.ActivationFunctionType
ALU = mybir.AluOpType

B_TOT = 16
N_CORES = 8
B_PER = B_TOT // N_CORES
CIN = 128
COUT = 128
H = W = 112
PW2 = 122                  # 7 lead + 112 + 3 trail cols
UROWS = 56                 # output rows per unit (half image)
XROWS = 76                 # 7 lead + 62 (56+halo) + 7 trail rows
XBF = XROWS * PW2          # 9272
DROWS = 59                 # valid x rows DMA'd per unit
QK = XROWS - 7             # 69 scanned values per column page
OUT_ROWS = 4
OTF = OUT_ROWS * W         # 448
TPG = 7                    # psum tiles per group
GPU = UROWS // (OUT_ROWS * TPG)  # 2 groups per unit
QTAP_V = (-3, -2, -1, 1, 2, 3)
NTAPS = 7 + len(QTAP_V)    # 13
# R-scan chunks (row-aligned; chunk 0 covers all rows group 1 needs)
RCHUNKS = ((7, 34), (41, 28))

_CACHE = {}


def _register_opa():
    from concourse.dve_spec import Spec, Src0, Src1, scan, AluOp, lower
    import concourse.dve_ops as dve_ops
    from concourse.dve_uop import DveOpSpec

    if any(op.name == "BOXDIFF7" for op in dve_ops.OPS):
        return next(op for op in dve_ops.OPS if op.name == "BOXDIFF7")
    spec = Spec(
        body=scan(AluOp.ADD, Src0) - scan(AluOp.ADD, Src1),
        reference=lambda in0, in1: (
            np.cumsum(in0, axis=-1) - np.cumsum(in1, axis=-1)
        ),
    )
    row = dve_ops._CUSTOM_DVE_ROW_BASE + len(dve_ops.OPS)
    shas = {}
    for ver in ("v3", "v4"):
        s = DveOpSpec(
            name="BOXDIFF7", opcode=row, uops=lower(spec, ver=ver), rd1_en=True
        )
        shas[ver] = s.sha(ver)
    op = dve_ops.DveOp("BOXDIFF7", spec, subdim=False, uops_sha=shas)
    dve_ops.OPS.append(op)
    dve_ops._SUB_OPCODE_FOR_NAME[op.name] = row
    dve_ops.CUSTOM_DVE_SPECS[op.name] = op.spec
    return op


def _build():
    nc = bacc.Bacc("TRN2", target_bir_lowering=False, debug=False)
    opa = _register_opa()

    t_x = nc.dram_tensor("xs", [B_PER, CIN, H, W], F32, kind="ExternalInput")
    t_w = nc.dram_tensor("wts", [NTAPS, CIN, COUT], F32, kind="ExternalInput")
    t_bias = nc.dram_tensor("bias", [COUT, 1], F32, kind="ExternalInput")
    t_out = nc.dram_tensor("out", [B_PER, COUT, H, W], BF16, kind="ExternalOutput")

    with tile.TileContext(nc) as tc:
        with (
            tc.tile_pool(name="const", bufs=1) as cpool,
            tc.tile_pool(name="bufs", bufs=1) as bpool,
            tc.tile_pool(name="outs", bufs=4) as opool,
            tc.tile_pool(name="ps", bufs=1, space="PSUM") as ppool,
        ):
            # ---- constants ----
            wf = cpool.tile([CIN, NTAPS * COUT], F32, tag="wf")
            nc.sync.dma_start(
                wf[:].rearrange("c (t o) -> c t o", t=NTAPS),
                t_w[:].transpose([1, 0, 2]),
            )
            wt = cpool.tile([CIN, NTAPS * COUT], BF16, tag="wt")
            nc.vector.tensor_copy(wt[:], wf[:])
            bias_sb = cpool.tile([COUT, 1], F32, tag="bias")
            nc.sync.dma_start(bias_sb[:], t_bias[:])

            # ---- per-slot buffers (slot = unit parity = top/bottom) ----
            xbufs, rbufs, qbufs = [], [], []
            for s in range(2):
                xb = bpool.tile([CIN, XBF], F32, tag=f"xb{s}", name=f"xb{s}")
                xv = xb[:].rearrange("c (r q) -> c r q", q=PW2)
                nc.vector.memset(xb[:, : 7 * PW2], 0.0)          # lead rows
                nc.vector.memset(xb[:, (XROWS - 7) * PW2 :], 0.0)  # trail rows
                nc.vector.memset(xv[:, 7 : XROWS - 7, 0:7], 0.0)   # lead cols
                nc.vector.memset(xv[:, 7 : XROWS - 7, 7 + W :], 0.0)  # trail cols
                if s == 0:
                    nc.vector.memset(xv[:, 7:10, :], 0.0)   # above-image pad
                else:
                    nc.vector.memset(xv[:, 66:69, :], 0.0)  # below-image pad
                xbufs.append(xb)
                r = bpool.tile([CIN, XBF], BF16, tag=f"R{s}", name=f"R{s}")
                rbufs.append(r)
                # Qp: scan output, page(col)-major contiguous.
                # Qg: row-major relayout [56 k-rows x 118 cols], cols = gc 4..121.
                qp = bpool.tile([CIN, 115 * QK], BF16, tag=f"Qp{s}", name=f"Qp{s}")
                qg = bpool.tile([CIN, UROWS * 118], BF16, tag=f"Qg{s}", name=f"Qg{s}")
                qgv = qg[:].rearrange("c (k g) -> c k g", g=118)
                nc.vector.memset(qgv[:, :, 115:118], 0.0)  # gc 119..121 zeros
                qbufs.append((qp, qg))

            for unit in range(B_PER * 2):
                b, hh = unit // 2, unit % 2
                xb, R = xbufs[hh], rbufs[hh]
                qp, qg = qbufs[hh]
                xv = xb[:].rearrange("c (r q) -> c r q", q=PW2)
                xt = xb[:].rearrange("c (r q) -> c q r", q=PW2)  # [c,122,76]
                rv = R[:].rearrange("c (r q) -> c r q", q=PW2)
                qgv = qg[:].rearrange("c (k g) -> c k g", g=118)

                # ---- load unit rows (2 chunks aligned with R chunks) ----
                # x rows d0-3..d0+58 clipped to the image; slot parity fixes
                # the destination row offset.
                xr0 = max(0, 56 * hh - 3)            # first valid x row
                dst0 = 10 if hh == 0 else 7          # its XB row
                # chunk split at XB row 41 (exclusive)
                n0 = 41 - dst0
                nc.sync.dma_start(
                    xv[:, dst0 : dst0 + n0, 7 : 7 + W],
                    t_x[b, :, xr0 : xr0 + n0, :],
                )
                n1 = DROWS - n0
                nc.sync.dma_start(
                    xv[:, 41 : 41 + n1, 7 : 7 + W],
                    t_x[b, :, xr0 + n0 : xr0 + DROWS, :],
                )

                # ---- R chunk 0, Q scan, R chunk 1 ----
                def r_chunk(r0, nrows):
                    base = r0 * PW2
                    ln = nrows * PW2 - 7
                    nc.vector._custom_dve(
                        opa,
                        out=R[:, base : base + ln],
                        in0=xb[:, base + 7 : base + 7 + ln],
                        in1=xb[:, base : base + ln],
                    )

                r_chunk(*RCHUNKS[0])
                nc.vector._custom_dve(
                    opa,
                    out=qp[:].rearrange("c (p k) -> c p k", k=QK),
                    in0=xt[:, 4 : 4 + 115, 7:XROWS],
                    in1=xt[:, 4 : 4 + 115, 0:QK],
                )
                r_chunk(*RCHUNKS[1])
                # relayout Qp (col-major) -> Qg (row-major) on the scalar engine
                nc.scalar.copy(
                    qgv[:, :, 0:115],
                    qp[:].rearrange("c (p k) -> c k p", k=QK)[:, 6 : 6 + UROWS, :],
                )

                # ---- 13-tap matmuls, weight-major per 7-tile group ----
                for g in range(GPU):
                    accs = [
                        ppool.tile([COUT, OTF], F32, tag=f"acc{t}", name=f"acc{t}")
                        for t in range(TPG)
                    ]
                    for tap in range(NTAPS):
                        wslice = wt[:, tap * COUT : (tap + 1) * COUT]
                        for t in range(TPG):
                            i0 = (g * TPG + t) * OUT_ROWS  # unit-local out row
                            if tap < 7:
                                u = tap - 3
                                rhs = rv[:, 10 + i0 + u : 14 + i0 + u, 3 : 3 + W]
                            else:
                                v = QTAP_V[tap - 7]
                                rhs = qgv[:, i0 : i0 + 4, 3 + v : 3 + v + 112]
                            nc.tensor.matmul(
                                accs[t][:],
                                wslice,
                                rhs,
                                start=(tap == 0),
                                stop=(tap == NTAPS - 1),
                            )
                    for t in range(TPG):
                        i0 = (g * TPG + t) * OUT_ROWS
                        ot = opool.tile([COUT, OTF], BF16, tag="ot", name="ot")
                        nc.scalar.activation(
                            ot[:], accs[t][:], AF.Identity, bias=bias_sb[:], scale=1.0
                        )
                        nc.sync.dma_start(
                            t_out[
                                b, :, 56 * hh + i0 : 56 * hh + i0 + OUT_ROWS, :
                            ].rearrange("o r j -> o (r j)"),
                            ot[:],
                        )

    nc.compile()
    return nc


def _make_in_maps(x, abc, bias):
    A, Bm, Cc = abc[0:128], abc[128:256], abc[256:384]
    taps = [u * A + Cc for u in range(-3, 4)] + [v * Bm for v in QTAP_V]
    wts = np.ascontiguousarray(np.stack(taps), dtype=np.float32)
    bias2 = np.ascontiguousarray(bias.reshape(COUT, 1), dtype=np.float32)
    return [
        {
            "xs": np.ascontiguousarray(x[c * B_PER : (c + 1) * B_PER]),
            "wts": wts,
            "bias": bias2,
        }
        for c in range(N_CORES)
    ]


def kernel(x: np.ndarray, abc: np.ndarray, bias: np.ndarray) -> np.ndarray:
    x = np.ascontiguousarray(x, dtype=np.float32)
    abc = np.asarray(abc, dtype=np.float32)
    bias = np.asarray(bias, dtype=np.float32)

    if "nc" not in _CACHE:
        _CACHE["nc"] = _build()
    nc = _CACHE["nc"]

    in_maps = _make_in_maps(x, abc, bias)
    res = run_bass_kernel_spmd(nc, in_maps, list(range(N_CORES)))
    out = np.concatenate(
        [np.asarray(res.results[c]["out"]) for c in range(N_CORES)], axis=0
    )
    return out.astype(np.float32)


if __name__ == "__main__":
    rng = np.random.default_rng(0)
    x = rng.standard_normal((16, 128, 112, 112), dtype=np.float32)
    abc = (rng.standard_normal((384, 128)) * 0.05).astype(np.float32)
    bias = (rng.standard_normal((128,)) * 0.05).astype(np.float32)
    out = kernel(x=x, abc=abc, bias=bias)
    print(out.shape, out.dtype)


# revision 10
# speedup vs baseline: 2.1098x; 1.0794x over previous
"""LConv (7x7 position-linear conv) Trainium2 Bass kernel.

Full inputs in, full output out. Sharding: data-parallel over batch,
16 images -> 8 NeuronCores (2 images/core). abc/bias replicated.

Math: the 7x7 kernel weight is linear in position:
  w[u,v,c,o] = u*A[c,o] + v*B[c,o] + C[c,o]   (u,v in -3..3)
so with R = box7 along W of x and Q = box7 along H of x:
  out[o,i,j] = sum_u (u*A + C)[.,o] . R[., i+u, j]
             + sum_{v!=0} (v*B)[.,o] . Q[., i, j+v]  + bias[o]
13 matmul taps per 4-row output tile over two box-filtered maps; all
rhs views are row-contiguous (PE streams at ~N cycles per matmul).

R: sliding-box via the BOXDIFF custom-DVE op (cumsum(in0)-cumsum(in1))
   on the row-major stream; 7 lead zero cols per row make any
   row-aligned chunk self-contained.
Q: same op on a column-major (transposed) view, writing into a
   row-major Q buffer; 7 lead + 7 trail zero rows make every column
   page self-telescoping.

The image pair is processed as 4 half-image units (56 out rows each,
+/-3-row halo) so DMA, scans, and matmuls pipeline across units;
unit slot parity == top/bottom parity, so the static zero borders per
slot are set up once.
"""

import numpy as np

import concourse.bacc as bacc
import concourse.mybir as mybir
from concourse import tile
from concourse.bass_utils import run_bass_kernel_spmd

F32 = mybir.dt.float32
BF16 = mybir.dt.bfloat16
AF = mybir.ActivationFunctionType
ALU = mybir.AluOpType

B_TOT = 16
N_CORES = 8
B_PER = B_TOT // N_CORES
CIN = 128
COUT = 128
H = W = 112
PW2 = 122                  # 7 lead + 112 + 3 trail cols
UROWS = 56                 # output rows per unit (half image)
XROWS = 76                 # 7 lead + 62 (56+halo) + 7 trail rows
XBF = XROWS * PW2          # 9272
DROWS = 59                 # valid x rows DMA'd per unit
QK = XROWS - 7             # 69 scanned values per column page
OUT_ROWS = 4
OTF = OUT_ROWS * W         # 448
TPG = 7                    # psum tiles per group
GPU = UROWS // (OUT_ROWS * TPG)  # 2 groups per unit
QTAP_V = (-3, -2, -1, 1, 2, 3)
NTAPS = 7 + len(QTAP_V)    # 13
# R-scan chunks (row-aligned; chunk 0 covers all rows group 1 needs)
RCHUNKS = ((7, 34), (41, 28))

_CACHE = {}


def _register_opa():
    from concourse.dve_spec import Spec, Src0, Src1, scan, AluOp, lower
    import concourse.dve_ops as dve_ops
    from concourse.dve_uop import DveOpSpec

    if any(op.name == "BOXDIFF7" for op in dve_ops.OPS):
        return next(op for op in dve_ops.OPS if op.name == "BOXDIFF7")
    spec = Spec(
        body=scan(AluOp.ADD, Src0) - scan(AluOp.ADD, Src1),
        reference=lambda in0, in1: (
            np.cumsum(in0, axis=-1) - np.cumsum(in1, axis=-1)
        ),
    )
    row = dve_ops._CUSTOM_DVE_ROW_BASE + len(dve_ops.OPS)
    shas = {}
    for ver in ("v3", "v4"):
        s = DveOpSpec(
            name="BOXDIFF7", opcode=row, uops=lower(spec, ver=ver), rd1_en=True
        )
        shas[ver] = s.sha(ver)
    op = dve_ops.DveOp("BOXDIFF7", spec, subdim=False, uops_sha=shas)
    dve_ops.OPS.append(op)
    dve_ops._SUB_OPCODE_FOR_NAME[op.name] = row
    dve_ops.CUSTOM_DVE_SPECS[op.name] = op.spec
    return op


def _build():
    nc = bacc.Bacc("TRN2", target_bir_lowering=False, debug=False)
    opa = _register_opa()

    t_x = nc.dram_tensor("xs", [B_PER, CIN, H, W], F32, kind="ExternalInput")
    t_w = nc.dram_tensor("wts", [NTAPS, CIN, COUT], F32, kind="ExternalInput")
    t_bias = nc.dram_tensor("bias", [COUT, 1], F32, kind="ExternalInput")
    t_out = nc.dram_tensor("out", [B_PER, COUT, H, W], BF16, kind="ExternalOutput")

    with tile.TileContext(nc) as tc:
        with (
            tc.tile_pool(name="const", bufs=1) as cpool,
            tc.tile_pool(name="bufs", bufs=1) as bpool,
            tc.tile_pool(name="outs", bufs=4) as opool,
            tc.tile_pool(name="ps", bufs=1, space="PSUM") as ppool,
        ):
            # ---- constants ----
            wf = cpool.tile([CIN, NTAPS * COUT], F32, tag="wf")
            nc.sync.dma_start(
                wf[:].rearrange("c (t o) -> c t o", t=NTAPS),
                t_w[:].transpose([1, 0, 2]),
            )
            wt = cpool.tile([CIN, NTAPS * COUT], BF16, tag="wt")
            nc.vector.tensor_copy(wt[:], wf[:])
            bias_sb = cpool.tile([COUT, 1], F32, tag="bias")
            nc.sync.dma_start(bias_sb[:], t_bias[:])

            # ---- per-slot buffers (slot = unit parity = top/bottom) ----
            xbufs, rbufs, qbufs = [], [], []
            for s in range(2):
                xb = bpool.tile([CIN, XBF], F32, tag=f"xb{s}", name=f"xb{s}")
                xv = xb[:].rearrange("c (r q) -> c r q", q=PW2)
                nc.gpsimd.memset(xb[:, : 7 * PW2], 0.0)          # lead rows
                nc.gpsimd.memset(xb[:, (XROWS - 7) * PW2 :], 0.0)  # trail rows
                nc.gpsimd.memset(xv[:, 7 : XROWS - 7, 0:7], 0.0)   # lead cols
                nc.gpsimd.memset(xv[:, 7 : XROWS - 7, 7 + W :], 0.0)  # trail cols
                if s == 0:
                    nc.gpsimd.memset(xv[:, 7:10, :], 0.0)   # above-image pad
                else:
                    nc.gpsimd.memset(xv[:, 66:69, :], 0.0)  # below-image pad
                xbufs.append(xb)
                r = bpool.tile([CIN, XBF], BF16, tag=f"R{s}", name=f"R{s}")
                rbufs.append(r)
                # Qp: scan output, page(col)-major contiguous.
                # Qg: row-major relayout [56 k-rows x 118 cols], cols = gc 4..121.
                qp = bpool.tile([CIN, 115 * QK], BF16, tag=f"Qp{s}", name=f"Qp{s}")
                qg = bpool.tile([CIN, UROWS * 118], BF16, tag=f"Qg{s}", name=f"Qg{s}")
                qgv = qg[:].rearrange("c (k g) -> c k g", g=118)
                nc.gpsimd.memset(qgv[:, :, 115:118], 0.0)  # gc 119..121 zeros
                qbufs.append((qp, qg))

            for unit in range(B_PER * 2):
                b, hh = unit // 2, unit % 2
                xb, R = xbufs[hh], rbufs[hh]
                qp, qg = qbufs[hh]
                xv = xb[:].rearrange("c (r q) -> c r q", q=PW2)
                xt = xb[:].rearrange("c (r q) -> c q r", q=PW2)  # [c,122,76]
                rv = R[:].rearrange("c (r q) -> c r q", q=PW2)
                qgv = qg[:].rearrange("c (k g) -> c k g", g=118)

                # ---- load unit rows (2 chunks aligned with R chunks) ----
                # x rows d0-3..d0+58 clipped to the image; slot parity fixes
                # the destination row offset.
                xr0 = max(0, 56 * hh - 3)            # first valid x row
                dst0 = 10 if hh == 0 else 7          # its XB row
                # chunk split at XB row 41 (exclusive)
                n0 = 41 - dst0
                n1 = DROWS - n0
                for s0, ln in ((0, n0 // 2), (n0 // 2, n0 - n0 // 2),
                               (n0, n1 // 2), (n0 + n1 // 2, n1 - n1 // 2)):
                    nc.sync.dma_start(
                        xv[:, dst0 + s0 : dst0 + s0 + ln, 7 : 7 + W],
                        t_x[b, :, xr0 + s0 : xr0 + s0 + ln, :],
                    )

                # ---- R chunk 0, Q scan, R chunk 1 ----
                def r_chunk(r0, nrows):
                    base = r0 * PW2
                    ln = nrows * PW2 - 7
                    nc.vector._custom_dve(
                        opa,
                        out=R[:, base : base + ln],
                        in0=xb[:, base + 7 : base + 7 + ln],
                        in1=xb[:, base : base + ln],
                    )

                r_chunk(*RCHUNKS[0])
                nc.vector._custom_dve(
                    opa,
                    out=qp[:].rearrange("c (p k) -> c p k", k=QK),
                    in0=xt[:, 4 : 4 + 115, 7:XROWS],
                    in1=xt[:, 4 : 4 + 115, 0:QK],
                )
                r_chunk(*RCHUNKS[1])
                # relayout Qp (col-major) -> Qg (row-major) on the scalar engine
                qpk = qp[:].rearrange("c (p k) -> c k p", k=QK)
                nc.scalar.copy(qgv[:, 0:28, 0:115], qpk[:, 6:34, :])
                nc.scalar.copy(qgv[:, 28:56, 0:115], qpk[:, 34:62, :])

                # ---- 13-tap matmuls, weight-major per 7-tile group ----
                for g in range(GPU):
                    accs = [
                        ppool.tile([COUT, OTF], F32, tag=f"acc{t}", name=f"acc{t}")
                        for t in range(TPG)
                    ]
                    for tap in range(NTAPS):
                        wslice = wt[:, tap * COUT : (tap + 1) * COUT]
                        for t in range(TPG):
                            i0 = (g * TPG + t) * OUT_ROWS  # unit-local out row
                            if tap < 7:
                                u = tap - 3
                                rhs = rv[:, 10 + i0 + u : 14 + i0 + u, 3 : 3 + W]
                            else:
                                v = QTAP_V[tap - 7]
                                rhs = qgv[:, i0 : i0 + 4, 3 + v : 3 + v + 112]
                            nc.tensor.matmul(
                                accs[t][:],
                                wslice,
                                rhs,
                                start=(tap == 0),
                                stop=(tap == NTAPS - 1),
                            )
                    for t in range(TPG):
                        i0 = (g * TPG + t) * OUT_ROWS
                        ot = opool.tile([COUT, OTF], BF16, tag="ot", name="ot")
                        nc.scalar.activation(
                            ot[:], accs[t][:], AF.Identity, bias=bias_sb[:], scale=1.0
                        )
                        nc.sync.dma_start(
                            t_out[
                                b, :, 56 * hh + i0 : 56 * hh + i0 + OUT_ROWS, :
                            ].rearrange("o r j -> o (r j)"),
                            ot[:],
                        )

    nc.compile()
    return nc


def _make_in_maps(x, abc, bias):
    A, Bm, Cc = abc[0:128], abc[128:256], abc[256:384]
    taps = [u * A + Cc for u in range(-3, 4)] + [v * Bm for v in QTAP_V]
    wts = np.ascontiguousarray(np.stack(taps), dtype=np.float32)
    bias2 = np.ascontiguousarray(bias.reshape(COUT, 1), dtype=np.float32)
    return [
        {
            "xs": np.ascontiguousarray(x[c * B_PER : (c + 1) * B_PER]),
            "wts": wts,
            "bias": bias2,
        }
        for c in range(N_CORES)
    ]


def kernel(x: np.ndarray, abc: np.ndarray, bias: np.ndarray) -> np.ndarray:
    x = np.ascontiguousarray(x, dtype=np.float32)
    abc = np.asarray(abc, dtype=np.float32)
    bias = np.asarray(bias, dtype=np.float32)

    if "nc" not in _CACHE:
        _CACHE["nc"] = _build()
    nc = _CACHE["nc"]

    in_maps = _make_in_maps(x, abc, bias)
    res = run_bass_kernel_spmd(nc, in_maps, list(range(N_CORES)))
    out = np.concatenate(
        [np.asarray(res.results[c]["out"]) for c in range(N_CORES)], axis=0
    )
    return out.astype(np.float32)


if __name__ == "__main__":
    rng = np.random.default_rng(0)
    x = rng.standard_normal((16, 128, 112, 112), dtype=np.float32)
    abc = (rng.standard_normal((384, 128)) * 0.05).astype(np.float32)
    bias = (rng.standard_normal((128,)) * 0.05).astype(np.float32)
    out = kernel(x=x, abc=abc, bias=bias)
    print(out.shape, out.dtype)
